# revision 11
# baseline (speedup 1.0000x reference)
"""Cross-attention without softmax on 8 trn2 NeuronCores.

Reference computes out = (X Wq^T) (C Wk^T)^T (C Wv^T) * D^-0.5 per batch.
With no softmax the product reassociates:

    out_b = X_b @ A_b,   A_b = scale * Wq^T Wk (C_b^T C_b) Wv^T

which collapses the O(Sq*Skv*D) attention into two O(S*D^2) matmuls plus
a few 128x128 products. Sharding: batch (4) x query-half (2) -> 8 cores;
each core redundantly computes its batch's G = C^T C (no collectives).

v3: the host supplies X^T (and receives out^T), so the kernel never
transposes on-chip: out^T = A^T @ X^T with A as the PE-stationary
operand and X^T streamed 512 columns at a time.  Loads are spread over
all three HWDGE queues; ctx chunks are sized small at both ends (early
G start, short G tail).  The tensor engine's clock ramps to full speed
only after ~3us of continuous work, so filler matmuls bridge the DMA
inter-arrival gaps and keep the ramp alive.  The last ctx chunk
accumulates into its own PSUM region so the main G copy and first half
of P = G Wv^T run off the critical path.  PSUM->SBUF casts go on
Vector+Scalar, stores on Sync+GpSimd.

I/O is bf16 (halves HBM traffic); accumulation stays fp32 in PSUM.
ctx row-tiles use a permuted grouping (partition p holds DRAM rows
p*r+j) so every DMA moves >=512B contiguous per partition; G's row-sum
is invariant to that permutation.
"""

import os
import sys
import types

import numpy as np

_TRN_REPO = "/opt/trn_rl_repo"
if _TRN_REPO not in sys.path and not any("trn_rl_repo" in p for p in sys.path):
    sys.path.insert(0, _TRN_REPO)

import ml_dtypes  # noqa: E402

import concourse.bass as bass  # noqa: E402
import concourse.mybir as mybir  # noqa: E402
from concourse import bacc  # noqa: E402
from concourse.bass_utils import run_bass_kernel_spmd  # noqa: E402

B, SQ, SKV, D = 4, 4096, 4096, 128
N_CORES = 8
SQ_SHARD = SQ // (N_CORES // B)  # 2048
SCALE = float(D) ** -0.5
F32 = mybir.dt.float32
BF16 = mybir.dt.bfloat16

# ctx chunk sizes in rows: small first chunk for an early G start, small
# last chunk for a short post-load G tail. Must sum to SKV. The last
# chunk is G_last (separate PSUM accumulation).
CTX_CHUNKS = [256, 768, 1024, 1024, 512, 512]
assert sum(CTX_CHUNKS) == SKV
# filler matmuls after each chunk's real matmuls (bridges DMA gaps to
# keep the PE p-state ramp alive)
FILLERS = [2, 4, 4, 2, 2, 0]

_CACHE: dict = {}


def _install_axon_ntff_shim():
    try:
        import antenv.axon_hooks  # noqa: F401

        return
    except Exception:
        pass
    try:
        from trn_agent_boot.trn_boot import _ntff_profile_via_ctypes

        import antenv

        hook = _ntff_profile_via_ctypes("/opt/axon/libaxon_pjrt.so")
        mod = types.ModuleType("antenv.axon_hooks")
        mod._hook = hook
        mod.get_axon_ntff_profile_hook = lambda: mod._hook

        def _set(h):
            mod._hook = h

        mod.set_axon_ntff_profile_hook = _set
        antenv.axon_hooks = mod
        sys.modules["antenv.axon_hooks"] = mod
    except Exception:
        pass

    try:
        import concourse.bass_utils as bu

        bu.upload_artifacts = lambda tmpdir: f"file://{tmpdir}"
    except Exception:
        pass


def build_v3():
    """Per-core inputs: xt = X_shard^T [128, 2048], ctx [4096, 128],
    w = [wq*scale | wk | wv^T] packed [128, 384]; output outt = out^T
    [128, 2048]. All bf16.

    PSUM banks: b0=G_main b1=UT(+G_last at [:,128:256]) b2=P b3=A
    b4..7=outT chunks (also filler scratch).
    """
    from contextlib import ExitStack

    cdt = BF16
    nc = bacc.Bacc(None, target_bir_lowering=False, debug=False)
    xt_ext = nc.declare_dram_parameter("xt", [D, SQ_SHARD], cdt, isOutput=False)
    c_ext = nc.declare_dram_parameter("ctx", [SKV, D], cdt, isOutput=False)
    w_ext = nc.declare_dram_parameter("w", [D, 3 * D], cdt, isOutput=False)
    outt_ext = nc.declare_dram_parameter(
        "outt", [D, SQ_SHARD], cdt, isOutput=True
    )

    ncc = len(CTX_CHUNKS)
    offs = [sum(CTX_CHUNKS[:i]) for i in range(ncc)]
    rpp = [n // 128 for n in CTX_CHUNKS]  # rows per partition per chunk
    ctx_view = [
        c_ext[offs[i] : offs[i] + CTX_CHUNKS[i], :].rearrange(
            "(p r) d -> p r d", p=128
        )
        for i in range(ncc)
    ]

    es = ExitStack()
    _n = [0]

    def sb(shape, dt, name=None):
        _n[0] += 1
        return es.enter_context(nc.sbuf_tensor(name or f"sb{_n[0]}", shape, dt))

    def pst(shape, dt, name=None):
        _n[0] += 1
        return es.enter_context(nc.psum_tensor(name or f"ps{_n[0]}", shape, dt))

    def sem(name):
        return es.enter_context(nc.semaphore(name))

    with es:
        w_sb = sb([D, 3 * D], cdt, "w_sb")
        cc = [sb([128, rpp[i], D], cdt, f"cc{i}") for i in range(ncc)]
        xt_sb = sb([D, SQ_SHARD], cdt, "xt_sb")
        ut_sb = sb([D, D], cdt, "ut_sb")
        gs_m = sb([D, D], cdt, "gs_m")
        gs_l = sb([D, D], cdt, "gs_l")
        ps_sb = sb([D, D], cdt, "ps_sb")
        a_sb = sb([D, D], cdt, "a_sb")
        o_sb = [sb([128, 512], cdt, f"o_sb{k}") for k in range(4)]

        g_ps = pst([128, 512], F32)  # b0 (use [:, :128])
        ut_ps = pst([128, 512], F32)  # b1: UT [:, :128], G_last [:, 128:256]
        p_ps = pst([128, 512], F32)  # b2
        a_ps = pst([128, 512], F32)  # b3
        o_ps = [pst([128, 512], F32) for _ in range(4)]  # b4..b7

        s_w = sem("s_w")
        s_x = sem("s_x")
        s_c = [sem(f"s_c{i}") for i in range(ncc)]
        s_pe = sem("s_pe")
        s_dve = sem("s_dve")
        s_o = [sem(f"s_o{k}") for k in range(4)]
        s_st = sem("s_st")

        # ---- cumulative s_pe schedule -------------------------------
        pe = [0]

        def inc():
            pe[0] += 1
            return pe[0]

        # chunk 0 real mms: 1..rpp0; UT; fillers; chunk1; ... computed
        # inline below via inc() so filler-count edits stay consistent.

        with nc.Block() as block:

            @block.sync
            def _(sync):
                nc.sync.dma_start(cc[0][:], ctx_view[0]).then_inc(s_c[0], 16)
                nc.sync.dma_start(cc[2][:], ctx_view[2]).then_inc(s_c[2], 16)
                nc.sync.dma_start(cc[4][:], ctx_view[4]).then_inc(s_c[4], 16)
                nc.sync.wait_ge(s_o[0], 1)
                nc.sync.dma_start(outt_ext[:, 0:512], o_sb[0][:]).then_inc(
                    s_st, 16
                )
                nc.sync.wait_ge(s_o[2], 1)
                nc.sync.dma_start(
                    outt_ext[:, 1024:1536], o_sb[2][:]
                ).then_inc(s_st, 16)
                nc.sync.wait_ge(s_st, 64)

            @block.scalar
            def _(sc):
                nc.scalar.dma_start(cc[1][:], ctx_view[1]).then_inc(s_c[1], 16)
                nc.scalar.dma_start(cc[3][:], ctx_view[3]).then_inc(s_c[3], 16)
                nc.scalar.dma_start(cc[5][:], ctx_view[5]).then_inc(s_c[5], 16)
                # casts for chunks 1 and 3 (s_pe thresholds patched below)
                nc.scalar.wait_ge(s_pe, PE_O1)
                nc.scalar.copy(o_sb[1][:], o_ps[1][:]).then_inc(s_o[1], 1)
                nc.scalar.wait_ge(s_pe, PE_O3)
                nc.scalar.copy(o_sb[3][:], o_ps[3][:]).then_inc(s_o[3], 1)

            @block.gpsimd
            def _(gp):
                nc.gpsimd.dma_start(w_sb[:], w_ext[:]).then_inc(s_w, 16)
                nc.gpsimd.wait_ge(s_c[2], 16)
                nc.gpsimd.dma_start(xt_sb[:], xt_ext[:]).then_inc(s_x, 16)
                nc.gpsimd.wait_ge(s_o[1], 1)
                nc.gpsimd.dma_start(
                    outt_ext[:, 512:1024], o_sb[1][:]
                ).then_inc(s_st, 16)
                nc.gpsimd.wait_ge(s_o[3], 1)
                nc.gpsimd.dma_start(
                    outt_ext[:, 1536:2048], o_sb[3][:]
                ).then_inc(s_st, 16)
                nc.gpsimd.wait_ge(s_st, 64)

            @block.tensor
            def _(te):
                def filler(n, src):
                    # keep the PE busy across DMA gaps; results discarded
                    for i in range(n):
                        nc.tensor.matmul(
                            o_ps[i % 4][:, :128],
                            src,
                            src,
                            start=True,
                            stop=True,
                        ).then_inc(s_pe, 1)
                        inc()

                marks = {}
                # G over chunks 0..4 -> g_ps (G_main); chunk 5 -> ut_ps
                for c in range(ncc):
                    last_main = c == ncc - 2
                    is_last = c == ncc - 1
                    if is_last:
                        # b1 holds UT until the ut copy is done
                        nc.tensor.wait_ge(s_dve, 1)
                    nc.tensor.wait_ge(s_c[c], 16)
                    for j in range(rpp[c]):
                        dst = (
                            ut_ps[:, 128:256] if is_last else g_ps[:, :128]
                        )
                        nc.tensor.matmul(
                            dst,
                            cc[c][:, j, :],
                            cc[c][:, j, :],
                            start=(c == 0 and j == 0)
                            or (is_last and j == 0),
                            stop=(last_main and j == rpp[c] - 1)
                            or (is_last and j == rpp[c] - 1),
                        ).then_inc(s_pe, 1)
                        inc()
                    if c == 0:
                        # UT = Wk^T (scale*Wq), early (b1)
                        nc.tensor.wait_ge(s_w, 16)
                        nc.tensor.matmul(
                            ut_ps[:, :128],
                            w_sb[:, 128:256],
                            w_sb[:, 0:128],
                            start=True,
                            stop=True,
                        ).then_inc(s_pe, 1)
                        marks["ut"] = inc()
                    if last_main:
                        marks["gmain"] = pe[0]
                    if is_last:
                        marks["glast"] = pe[0]
                    filler(FILLERS[c], cc[c][:, 0, :])

                # P = G Wv^T split into main+last accumulation (b2)
                nc.tensor.wait_ge(s_x, 16)
                nc.tensor.wait_ge(s_dve, 2)
                nc.tensor.matmul(
                    p_ps[:, :128],
                    gs_m[:],
                    w_sb[:, 256:384],
                    start=True,
                    stop=False,
                ).then_inc(s_pe, 1)
                inc()
                nc.tensor.wait_ge(s_dve, 3)
                nc.tensor.matmul(
                    p_ps[:, :128],
                    gs_l[:],
                    w_sb[:, 256:384],
                    start=False,
                    stop=True,
                ).then_inc(s_pe, 1)
                marks["p"] = inc()
                # A = U P  (lhsT = U^T) (b3)
                nc.tensor.wait_ge(s_dve, 4)
                nc.tensor.matmul(
                    a_ps[:, :128], ut_sb[:], ps_sb[:], start=True, stop=True
                ).then_inc(s_pe, 1)
                marks["a"] = inc()
                # out^T = A^T X^T in 4 chunks of 512 query columns
                nc.tensor.wait_ge(s_dve, 5)
                for k in range(4):
                    nc.tensor.matmul(
                        o_ps[k][:],
                        a_sb[:],
                        xt_sb[:, 512 * k : 512 * (k + 1)],
                        start=True,
                        stop=True,
                    ).then_inc(s_pe, 1)
                    marks[f"o{k}"] = inc()
                MARKS.update(marks)

            @block.vector
            def _(ve):
                nc.vector.wait_ge(s_pe, MARKS["ut"])
                nc.vector.tensor_copy(ut_sb[:], ut_ps[:, :128]).then_inc(
                    s_dve, 1
                )
                nc.vector.wait_ge(s_pe, MARKS["gmain"])
                nc.vector.tensor_copy(gs_m[:], g_ps[:, :128]).then_inc(
                    s_dve, 1
                )
                nc.vector.wait_ge(s_pe, MARKS["glast"])
                nc.vector.tensor_copy(gs_l[:], ut_ps[:, 128:256]).then_inc(
                    s_dve, 1
                )
                nc.vector.wait_ge(s_pe, MARKS["p"])
                nc.vector.tensor_copy(ps_sb[:], p_ps[:, :128]).then_inc(
                    s_dve, 1
                )
                nc.vector.wait_ge(s_pe, MARKS["a"])
                nc.vector.tensor_copy(a_sb[:], a_ps[:, :128]).then_inc(
                    s_dve, 1
                )
                nc.vector.wait_ge(s_pe, MARKS["o0"])
                nc.vector.tensor_copy(o_sb[0][:], o_ps[0][:]).then_inc(
                    s_o[0], 1
                )
                nc.vector.wait_ge(s_pe, MARKS["o2"])
                nc.vector.tensor_copy(o_sb[2][:], o_ps[2][:]).then_inc(
                    s_o[2], 1
                )

    nc.compile()
    return nc


# s_pe marks filled in by the tensor section at build time; the scalar
# section needs two of them before the tensor section runs, so compute
# them statically here: every matmul (real, UT, filler, chain, out)
# increments s_pe by 1 in program order.
def _static_marks():
    pe = 0
    marks = {}
    rpp = [n // 128 for n in CTX_CHUNKS]
    ncc = len(CTX_CHUNKS)
    for c in range(ncc):
        pe += rpp[c]
        if c == 0:
            pe += 1
            marks["ut"] = pe
        if c == ncc - 2:
            marks["gmain"] = pe
        if c == ncc - 1:
            marks["glast"] = pe
        pe += FILLERS[c]
    pe += 1  # P_main
    pe += 1
    marks["p"] = pe
    pe += 1
    marks["a"] = pe
    for k in range(4):
        pe += 1
        marks[f"o{k}"] = pe
    return marks


MARKS = _static_marks()
PE_O1 = MARKS["o1"]
PE_O3 = MARKS["o3"]


def build():
    return build_v3()


def _get_nc():
    if "nc" not in _CACHE:
        _CACHE["nc"] = build()
    return _CACHE["nc"]


def _run(inputs: dict, trace: bool = False, **kw):
    np_dt = ml_dtypes.bfloat16
    context = np.ascontiguousarray(inputs["context"]).astype(np_dt)
    Wq = np.asarray(inputs["Wq"], dtype=np.float32) * SCALE
    Wk = np.asarray(inputs["Wk"], dtype=np.float32)
    Wvt = np.asarray(inputs["Wv"], dtype=np.float32).T
    w_pack = np.ascontiguousarray(
        np.concatenate([Wq, Wk, Wvt], axis=1)
    ).astype(np_dt)
    X = np.asarray(inputs["X"], dtype=np.float32)

    in_maps = []
    for c in range(N_CORES):
        b, h = divmod(c, 2)
        xt = np.ascontiguousarray(
            X[b, h * SQ_SHARD : (h + 1) * SQ_SHARD, :].T
        ).astype(np_dt)
        in_maps.append({"xt": xt, "ctx": context[b], "w": w_pack})

    nc = _get_nc()
    res = run_bass_kernel_spmd(
        nc, in_maps, core_ids=list(range(N_CORES)), trace=trace, **kw
    )
    out = np.empty((B, SQ, D), dtype=np.float32)
    for c in range(N_CORES):
        b, h = divmod(c, 2)
        out[b, h * SQ_SHARD : (h + 1) * SQ_SHARD, :] = (
            res.results[c]["outt"].astype(np.float32).T
        )
    return out, res


def kernel(**inputs: np.ndarray) -> np.ndarray:
    if os.environ.get("BASS_TRACE"):
        _install_axon_ntff_shim()
    try:
        out, _ = _run(inputs, trace=False)
    except Exception:
        # transient NRT device errors have been observed once across many
        # runs; one retry on a fresh execution
        out, _ = _run(inputs, trace=False)
    return out


if __name__ == "__main__":
    rng = np.random.default_rng(0)
    ins = {
        "context": rng.standard_normal((B, SKV, D)).astype(np.float32),
        "X": rng.standard_normal((B, SQ, D)).astype(np.float32),
        "Wq": (rng.standard_normal((D, D)) / np.sqrt(D)).astype(np.float32),
        "Wk": (rng.standard_normal((D, D)) / np.sqrt(D)).astype(np.float32),
        "Wv": (rng.standard_normal((D, D)) / np.sqrt(D)).astype(np.float32),
    }
    got = kernel(**ins)
    q = ins["X"] @ ins["Wq"].T
    k = ins["context"] @ ins["Wk"].T
    v = ins["context"] @ ins["Wv"].T
    w = np.einsum("bse,bte->bst", q, k) * SCALE
    want = np.einsum("bst,bte->bse", w, v)
    rel = np.linalg.norm(got - want) / np.linalg.norm(want)
    print("rel err vs numpy:", rel)


# revision 12
# speedup vs baseline: 1.0738x; 1.0738x over previous
"""Cross-attention without softmax on 8 trn2 NeuronCores.

Reference computes out = (X Wq^T) (C Wk^T)^T (C Wv^T) * D^-0.5 per batch.
With no softmax the product reassociates:

    out_b = X_b @ A_b,   A_b = scale * Wq^T Wk (C_b^T C_b) Wv^T

which collapses the O(Sq*Skv*D) attention into two O(S*D^2) matmuls plus
a few 128x128 products. Sharding: batch (4) x query-half (2) -> 8 cores;
each core redundantly computes its batch's G = C^T C (no collectives).

v4: the host supplies X^T (and receives out^T), so the kernel never
transposes on-chip: out^T = A^T @ X^T with A as the PE-stationary
operand and X^T streamed 512 columns at a time.  All DMAs ride the two
HWDGE queues (sync+scalar) — gpsimd DMA is software-DGE with ~2us
startup and laggy completion semaphores.  The weight pack is split
across both queues as a warmup transfer.  ctx chunks are sized small at
both ends (early G start, short G tail).  The tensor engine's clock
ramps to full speed only after ~3us of continuous work, so a few filler
matmuls bridge the early DMA inter-arrival gaps and keep the ramp
alive.  The last ctx chunk accumulates into its own PSUM region so the
main G copy and first half of P = G Wv^T run off the critical path.
Output casts go on Vector+Scalar into one [128,2048] tile, stored with
two 256KB DMAs from sync.

I/O is bf16 (halves HBM traffic); accumulation stays fp32 in PSUM.
ctx row-tiles use a permuted grouping (partition p holds DRAM rows
p*r+j) so every DMA moves >=512B contiguous per partition; G's row-sum
is invariant to that permutation.
"""

import os
import sys
import types

import numpy as np

_TRN_REPO = "/opt/trn_rl_repo"
if _TRN_REPO not in sys.path and not any("trn_rl_repo" in p for p in sys.path):
    sys.path.insert(0, _TRN_REPO)

import ml_dtypes  # noqa: E402

import concourse.bass as bass  # noqa: E402
import concourse.mybir as mybir  # noqa: E402
from concourse import bacc  # noqa: E402
from concourse.bass_utils import run_bass_kernel_spmd  # noqa: E402

B, SQ, SKV, D = 4, 4096, 4096, 128
N_CORES = 8
SQ_SHARD = SQ // (N_CORES // B)  # 2048
SCALE = float(D) ** -0.5
F32 = mybir.dt.float32
BF16 = mybir.dt.bfloat16

# ctx chunk sizes in rows: small first chunk for an early G start, small
# last chunk for a short post-load G tail. Must sum to SKV. The last
# chunk is G_last (separate PSUM accumulation).
CTX_CHUNKS = [256, 768, 1024, 1024, 512, 512]
assert sum(CTX_CHUNKS) == SKV
# filler matmuls after each chunk's real matmuls (bridges DMA gaps to
# keep the PE p-state ramp alive)
FILLERS = [2, 3, 2, 0, 0, 0]

_CACHE: dict = {}


def _install_axon_ntff_shim():
    try:
        import antenv.axon_hooks  # noqa: F401

        return
    except Exception:
        pass
    try:
        from trn_agent_boot.trn_boot import _ntff_profile_via_ctypes

        import antenv

        hook = _ntff_profile_via_ctypes("/opt/axon/libaxon_pjrt.so")
        mod = types.ModuleType("antenv.axon_hooks")
        mod._hook = hook
        mod.get_axon_ntff_profile_hook = lambda: mod._hook

        def _set(h):
            mod._hook = h

        mod.set_axon_ntff_profile_hook = _set
        antenv.axon_hooks = mod
        sys.modules["antenv.axon_hooks"] = mod
    except Exception:
        pass

    try:
        import concourse.bass_utils as bu

        bu.upload_artifacts = lambda tmpdir: f"file://{tmpdir}"
    except Exception:
        pass


# s_pe marks: every matmul (real, UT, filler, chain, out) increments
# s_pe by 1 in program order.
def _static_marks():
    pe = 0
    marks = {}
    rpp = [n // 128 for n in CTX_CHUNKS]
    ncc = len(CTX_CHUNKS)
    for c in range(ncc):
        pe += rpp[c]
        if c == 0:
            pe += 1
            marks["ut"] = pe
        if c == ncc - 2:
            marks["gmain"] = pe
        if c == ncc - 1:
            marks["glast"] = pe
        pe += FILLERS[c]
    pe += 1  # P_main
    pe += 1
    marks["p"] = pe
    pe += 1
    marks["a"] = pe
    for k in range(4):
        pe += 1
        marks[f"o{k}"] = pe
    return marks


MARKS = _static_marks()


def build_v4():
    """Per-core inputs: xt = X_shard^T [128, 2048], ctx [4096, 128],
    w = [wq*scale | wk | wv^T] packed [128, 384]; output outt = out^T
    [128, 2048]. All bf16.

    PSUM banks: b0=G_main b1=UT(+G_last at [:,128:256]) b2=P b3=A
    b4..7=outT chunks (also filler scratch).
    """
    from contextlib import ExitStack

    cdt = BF16
    nc = bacc.Bacc(None, target_bir_lowering=False, debug=False)
    xt_ext = nc.declare_dram_parameter("xt", [D, SQ_SHARD], cdt, isOutput=False)
    c_ext = nc.declare_dram_parameter("ctx", [SKV, D], cdt, isOutput=False)
    w_ext = nc.declare_dram_parameter("w", [D, 3 * D], cdt, isOutput=False)
    outt_ext = nc.declare_dram_parameter(
        "outt", [D, SQ_SHARD], cdt, isOutput=True
    )

    ncc = len(CTX_CHUNKS)
    offs = [sum(CTX_CHUNKS[:i]) for i in range(ncc)]
    rpp = [n // 128 for n in CTX_CHUNKS]  # rows per partition per chunk
    ctx_view = [
        c_ext[offs[i] : offs[i] + CTX_CHUNKS[i], :].rearrange(
            "(p r) d -> p r d", p=128
        )
        for i in range(ncc)
    ]

    es = ExitStack()
    _n = [0]

    def sb(shape, dt, name=None):
        _n[0] += 1
        return es.enter_context(nc.sbuf_tensor(name or f"sb{_n[0]}", shape, dt))

    def pst(shape, dt, name=None):
        _n[0] += 1
        return es.enter_context(nc.psum_tensor(name or f"ps{_n[0]}", shape, dt))

    def sem(name):
        return es.enter_context(nc.semaphore(name))

    with es:
        w_sb = sb([D, 3 * D], cdt, "w_sb")
        cc = [sb([128, rpp[i], D], cdt, f"cc{i}") for i in range(ncc)]
        xt_sb = sb([D, SQ_SHARD], cdt, "xt_sb")
        ut_sb = sb([D, D], cdt, "ut_sb")
        gs_m = sb([D, D], cdt, "gs_m")
        gs_l = sb([D, D], cdt, "gs_l")
        ps_sb = sb([D, D], cdt, "ps_sb")
        a_sb = sb([D, D], cdt, "a_sb")
        o_sb = sb([128, SQ_SHARD], cdt, "o_sb")

        g_ps = pst([128, 512], F32)  # b0 (use [:, :128])
        ut_ps = pst([128, 512], F32)  # b1: UT [:, :128], G_last [:, 128:256]
        p_ps = pst([128, 512], F32)  # b2
        a_ps = pst([128, 512], F32)  # b3
        o_ps = [pst([128, 512], F32) for _ in range(4)]  # b4..b7

        s_wa = sem("s_wa")  # wq|wk (sync)
        s_wb = sem("s_wb")  # wvt (scalar)
        s_x = sem("s_x")
        s_c = [sem(f"s_c{i}") for i in range(ncc)]
        s_pe = sem("s_pe")
        s_dve = sem("s_dve")
        s_o = [sem(f"s_o{k}") for k in range(4)]
        s_st = sem("s_st")

        pe = [0]

        def inc():
            pe[0] += 1
            return pe[0]

        with nc.Block() as block:

            @block.sync
            def _(sync):
                nc.sync.dma_start(w_sb[:, 0:256], w_ext[:, 0:256]).then_inc(
                    s_wa, 16
                )
                nc.sync.dma_start(cc[0][:], ctx_view[0]).then_inc(s_c[0], 16)
                nc.sync.dma_start(cc[2][:], ctx_view[2]).then_inc(s_c[2], 16)
                nc.sync.dma_start(cc[4][:], ctx_view[4]).then_inc(s_c[4], 16)
                nc.sync.wait_ge(s_o[0], 1)
                nc.sync.wait_ge(s_o[1], 1)
                nc.sync.dma_start(
                    outt_ext[:, 0:1024], o_sb[:, 0:1024]
                ).then_inc(s_st, 16)
                nc.sync.wait_ge(s_o[2], 1)
                nc.sync.wait_ge(s_o[3], 1)
                nc.sync.dma_start(
                    outt_ext[:, 1024:2048], o_sb[:, 1024:2048]
                ).then_inc(s_st, 16)
                nc.sync.wait_ge(s_st, 32)

            @block.scalar
            def _(sc):
                nc.scalar.dma_start(
                    w_sb[:, 256:384], w_ext[:, 256:384]
                ).then_inc(s_wb, 16)
                nc.scalar.dma_start(cc[1][:], ctx_view[1]).then_inc(s_c[1], 16)
                nc.scalar.dma_start(cc[3][:], ctx_view[3]).then_inc(s_c[3], 16)
                nc.scalar.dma_start(cc[5][:], ctx_view[5]).then_inc(s_c[5], 16)
                nc.scalar.dma_start(xt_sb[:], xt_ext[:]).then_inc(s_x, 16)
                nc.scalar.wait_ge(s_pe, MARKS["o1"])
                nc.scalar.copy(
                    o_sb[:, 512:1024], o_ps[1][:]
                ).then_inc(s_o[1], 1)
                nc.scalar.wait_ge(s_pe, MARKS["o3"])
                nc.scalar.copy(
                    o_sb[:, 1536:2048], o_ps[3][:]
                ).then_inc(s_o[3], 1)

            @block.gpsimd
            def _(gp):
                nc.gpsimd.wait_ge(s_st, 32)

            @block.tensor
            def _(te):
                def filler(n, src):
                    # keep the PE busy across DMA gaps; results discarded
                    for i in range(n):
                        nc.tensor.matmul(
                            o_ps[i % 4][:, :128],
                            src,
                            src,
                            start=True,
                            stop=True,
                        ).then_inc(s_pe, 1)
                        inc()

                marks = {}
                # G over chunks 0..4 -> g_ps (G_main); chunk 5 -> ut_ps
                for c in range(ncc):
                    last_main = c == ncc - 2
                    is_last = c == ncc - 1
                    if is_last:
                        # b1 holds UT until the ut copy is done
                        nc.tensor.wait_ge(s_dve, 1)
                    nc.tensor.wait_ge(s_c[c], 16)
                    for j in range(rpp[c]):
                        dst = (
                            ut_ps[:, 128:256] if is_last else g_ps[:, :128]
                        )
                        nc.tensor.matmul(
                            dst,
                            cc[c][:, j, :],
                            cc[c][:, j, :],
                            start=(c == 0 and j == 0)
                            or (is_last and j == 0),
                            stop=(last_main and j == rpp[c] - 1)
                            or (is_last and j == rpp[c] - 1),
                        ).then_inc(s_pe, 1)
                        inc()
                    if c == 0:
                        # UT = Wk^T (scale*Wq), early (b1)
                        nc.tensor.wait_ge(s_wa, 16)
                        nc.tensor.matmul(
                            ut_ps[:, :128],
                            w_sb[:, 128:256],
                            w_sb[:, 0:128],
                            start=True,
                            stop=True,
                        ).then_inc(s_pe, 1)
                        marks["ut"] = inc()
                    if last_main:
                        marks["gmain"] = pe[0]
                    if is_last:
                        marks["glast"] = pe[0]
                    filler(FILLERS[c], cc[c][:, 0, :])

                # P = G Wv^T split into main+last accumulation (b2)
                nc.tensor.wait_ge(s_x, 16)
                nc.tensor.wait_ge(s_wb, 16)
                nc.tensor.wait_ge(s_dve, 2)
                nc.tensor.matmul(
                    p_ps[:, :128],
                    gs_m[:],
                    w_sb[:, 256:384],
                    start=True,
                    stop=False,
                ).then_inc(s_pe, 1)
                inc()
                nc.tensor.wait_ge(s_dve, 3)
                nc.tensor.matmul(
                    p_ps[:, :128],
                    gs_l[:],
                    w_sb[:, 256:384],
                    start=False,
                    stop=True,
                ).then_inc(s_pe, 1)
                marks["p"] = inc()
                # A = U P  (lhsT = U^T) (b3)
                nc.tensor.wait_ge(s_dve, 4)
                nc.tensor.matmul(
                    a_ps[:, :128], ut_sb[:], ps_sb[:], start=True, stop=True
                ).then_inc(s_pe, 1)
                marks["a"] = inc()
                # out^T = A^T X^T in 4 chunks of 512 query columns
                nc.tensor.wait_ge(s_dve, 5)
                for k in range(4):
                    nc.tensor.matmul(
                        o_ps[k][:],
                        a_sb[:],
                        xt_sb[:, 512 * k : 512 * (k + 1)],
                        start=True,
                        stop=True,
                    ).then_inc(s_pe, 1)
                    marks[f"o{k}"] = inc()
                assert marks == MARKS, (marks, MARKS)

            @block.vector
            def _(ve):
                nc.vector.wait_ge(s_pe, MARKS["ut"])
                nc.vector.tensor_copy(ut_sb[:], ut_ps[:, :128]).then_inc(
                    s_dve, 1
                )
                nc.vector.wait_ge(s_pe, MARKS["gmain"])
                nc.vector.tensor_copy(gs_m[:], g_ps[:, :128]).then_inc(
                    s_dve, 1
                )
                nc.vector.wait_ge(s_pe, MARKS["glast"])
                nc.vector.tensor_copy(gs_l[:], ut_ps[:, 128:256]).then_inc(
                    s_dve, 1
                )
                nc.vector.wait_ge(s_pe, MARKS["p"])
                nc.vector.tensor_copy(ps_sb[:], p_ps[:, :128]).then_inc(
                    s_dve, 1
                )
                nc.vector.wait_ge(s_pe, MARKS["a"])
                nc.vector.tensor_copy(a_sb[:], a_ps[:, :128]).then_inc(
                    s_dve, 1
                )
                nc.vector.wait_ge(s_pe, MARKS["o0"])
                nc.vector.tensor_copy(o_sb[:, 0:512], o_ps[0][:]).then_inc(
                    s_o[0], 1
                )
                nc.vector.wait_ge(s_pe, MARKS["o2"])
                nc.vector.tensor_copy(
                    o_sb[:, 1024:1536], o_ps[2][:]
                ).then_inc(s_o[2], 1)

    nc.compile()
    return nc


def build():
    return build_v4()


def _get_nc():
    if "nc" not in _CACHE:
        _CACHE["nc"] = build()
    return _CACHE["nc"]


def _run(inputs: dict, trace: bool = False, **kw):
    np_dt = ml_dtypes.bfloat16
    context = np.ascontiguousarray(inputs["context"]).astype(np_dt)
    Wq = np.asarray(inputs["Wq"], dtype=np.float32) * SCALE
    Wk = np.asarray(inputs["Wk"], dtype=np.float32)
    Wvt = np.asarray(inputs["Wv"], dtype=np.float32).T
    w_pack = np.ascontiguousarray(
        np.concatenate([Wq, Wk, Wvt], axis=1)
    ).astype(np_dt)
    X = np.asarray(inputs["X"], dtype=np.float32)

    in_maps = []
    for c in range(N_CORES):
        b, h = divmod(c, 2)
        xt = np.ascontiguousarray(
            X[b, h * SQ_SHARD : (h + 1) * SQ_SHARD, :].T
        ).astype(np_dt)
        in_maps.append({"xt": xt, "ctx": context[b], "w": w_pack})

    nc = _get_nc()
    res = run_bass_kernel_spmd(
        nc, in_maps, core_ids=list(range(N_CORES)), trace=trace, **kw
    )
    out = np.empty((B, SQ, D), dtype=np.float32)
    for c in range(N_CORES):
        b, h = divmod(c, 2)
        out[b, h * SQ_SHARD : (h + 1) * SQ_SHARD, :] = (
            res.results[c]["outt"].astype(np.float32).T
        )
    return out, res


def kernel(**inputs: np.ndarray) -> np.ndarray:
    if os.environ.get("BASS_TRACE"):
        _install_axon_ntff_shim()
    try:
        out, _ = _run(inputs, trace=False)
    except Exception:
        # transient NRT device errors have been observed once across many
        # runs; one retry on a fresh execution
        out, _ = _run(inputs, trace=False)
    return out


if __name__ == "__main__":
    rng = np.random.default_rng(0)
    ins = {
        "context": rng.standard_normal((B, SKV, D)).astype(np.float32),
        "X": rng.standard_normal((B, SQ, D)).astype(np.float32),
        "Wq": (rng.standard_normal((D, D)) / np.sqrt(D)).astype(np.float32),
        "Wk": (rng.standard_normal((D, D)) / np.sqrt(D)).astype(np.float32),
        "Wv": (rng.standard_normal((D, D)) / np.sqrt(D)).astype(np.float32),
    }
    got = kernel(**ins)
    q = ins["X"] @ ins["Wq"].T
    k = ins["context"] @ ins["Wk"].T
    v = ins["context"] @ ins["Wv"].T
    w = np.einsum("bse,bte->bst", q, k) * SCALE
    want = np.einsum("bst,bte->bse", w, v)
    rel = np.linalg.norm(got - want) / np.linalg.norm(want)
    print("rel err vs numpy:", rel)


# revision 14
# speedup vs baseline: 1.1481x; 1.0692x over previous
"""Cross-attention without softmax on 8 trn2 NeuronCores.

Reference computes out = (X Wq^T) (C Wk^T)^T (C Wv^T) * D^-0.5 per batch.
With no softmax the product reassociates:

    out_b = X_b @ A_b,   A_b = scale * Wq^T Wk (C_b^T C_b) Wv^T

which collapses the O(Sq*Skv*D) attention into two O(S*D^2) matmuls plus
a few 128x128 products. Sharding: batch (4) x query-half (2) -> 8 cores;
each core redundantly computes its batch's G = C^T C (no collectives).

v4: the host supplies X^T (and receives out^T), so the kernel never
transposes on-chip: out^T = A^T @ X^T with A as the PE-stationary
operand and X^T streamed 512 columns at a time.  All DMAs ride the two
HWDGE queues (sync+scalar) — gpsimd DMA is software-DGE with ~2us
startup and laggy completion semaphores.  The weight pack is split
across both queues as a warmup transfer.  ctx chunks are sized small at
both ends (early G start, short G tail).  The tensor engine's clock
ramps to full speed only after ~3us of continuous work, so a few filler
matmuls bridge the early DMA inter-arrival gaps and keep the ramp
alive.  The last ctx chunk accumulates into its own PSUM region so the
main G copy and first half of P = G Wv^T run off the critical path.
Output casts go on Vector+Scalar into one [128,2048] tile, stored with
two 256KB DMAs from sync.

I/O is bf16 (halves HBM traffic); accumulation stays fp32 in PSUM.
ctx row-tiles use a permuted grouping (partition p holds DRAM rows
p*r+j) so every DMA moves >=512B contiguous per partition; G's row-sum
is invariant to that permutation.
"""

import os
import sys
import types

import numpy as np

_TRN_REPO = "/opt/trn_rl_repo"
if _TRN_REPO not in sys.path and not any("trn_rl_repo" in p for p in sys.path):
    sys.path.insert(0, _TRN_REPO)

import ml_dtypes  # noqa: E402

import concourse.bass as bass  # noqa: E402
import concourse.mybir as mybir  # noqa: E402
from concourse import bacc  # noqa: E402
from concourse.bass_utils import run_bass_kernel_spmd  # noqa: E402

B, SQ, SKV, D = 4, 4096, 4096, 128
N_CORES = 8
SQ_SHARD = SQ // (N_CORES // B)  # 2048
SCALE = float(D) ** -0.5
F32 = mybir.dt.float32
BF16 = mybir.dt.bfloat16

# ctx chunk sizes in rows: small first chunk for an early G start, small
# last chunk for a short post-load G tail. Must sum to SKV. The last
# chunk is G_last (separate PSUM accumulation).
CTX_CHUNKS = [256, 768, 1024, 1024, 512, 512]
assert sum(CTX_CHUNKS) == SKV
# filler matmuls after each chunk's real matmuls (bridges DMA gaps to
# keep the PE p-state ramp alive)
FILLERS = [2, 3, 2, 0, 0, 0]

_CACHE: dict = {}


def _install_axon_ntff_shim():
    try:
        import antenv.axon_hooks  # noqa: F401

        return
    except Exception:
        pass
    try:
        from trn_agent_boot.trn_boot import _ntff_profile_via_ctypes

        import antenv

        hook = _ntff_profile_via_ctypes("/opt/axon/libaxon_pjrt.so")
        mod = types.ModuleType("antenv.axon_hooks")
        mod._hook = hook
        mod.get_axon_ntff_profile_hook = lambda: mod._hook

        def _set(h):
            mod._hook = h

        mod.set_axon_ntff_profile_hook = _set
        antenv.axon_hooks = mod
        sys.modules["antenv.axon_hooks"] = mod
    except Exception:
        pass

    try:
        import concourse.bass_utils as bu

        bu.upload_artifacts = lambda tmpdir: f"file://{tmpdir}"
    except Exception:
        pass


# s_pe marks: every matmul (real, UT, filler, chain, out) increments
# s_pe by 1 in program order.
def _static_marks():
    pe = 0
    marks = {}
    rpp = [n // 128 for n in CTX_CHUNKS]
    ncc = len(CTX_CHUNKS)
    for c in range(ncc):
        pe += rpp[c]
        if c == 0:
            pe += 1
            marks["ut"] = pe
        if c == ncc - 2:
            marks["gmain"] = pe
        if c == ncc - 1:
            marks["glast"] = pe
        pe += FILLERS[c]
    pe += 1  # P_main
    pe += 1
    marks["p"] = pe
    pe += 1
    marks["a"] = pe
    for k in range(4):
        pe += 1
        marks[f"o{k}"] = pe
    return marks


MARKS = _static_marks()


def build_v4():
    """Per-core inputs: xt = X_shard^T [128, 2048], ctx [4096, 128],
    w = [wq*scale | wk | wv^T] packed [128, 384]; output outt = out^T
    [128, 2048]. All bf16.

    PSUM banks: b0=G_main b1=UT(+G_last at [:,128:256]) b2=P b3=A
    b4..7=outT chunks (also filler scratch).
    """
    from contextlib import ExitStack

    cdt = BF16
    nc = bacc.Bacc(None, target_bir_lowering=False, debug=False)
    xt_ext = nc.declare_dram_parameter("xt", [D, SQ_SHARD], cdt, isOutput=False)
    c_ext = nc.declare_dram_parameter("ctx", [SKV, D], cdt, isOutput=False)
    w_ext = nc.declare_dram_parameter("w", [D, 3 * D], cdt, isOutput=False)
    outt_ext = nc.declare_dram_parameter(
        "outt", [D, SQ_SHARD], cdt, isOutput=True
    )

    ncc = len(CTX_CHUNKS)
    offs = [sum(CTX_CHUNKS[:i]) for i in range(ncc)]
    rpp = [n // 128 for n in CTX_CHUNKS]  # rows per partition per chunk
    ctx_view = [
        c_ext[offs[i] : offs[i] + CTX_CHUNKS[i], :].rearrange(
            "(p r) d -> p r d", p=128
        )
        for i in range(ncc)
    ]

    es = ExitStack()
    _n = [0]

    def sb(shape, dt, name=None):
        _n[0] += 1
        return es.enter_context(nc.sbuf_tensor(name or f"sb{_n[0]}", shape, dt))

    def pst(shape, dt, name=None):
        _n[0] += 1
        return es.enter_context(nc.psum_tensor(name or f"ps{_n[0]}", shape, dt))

    def sem(name):
        return es.enter_context(nc.semaphore(name))

    with es:
        w_sb = sb([D, 3 * D], cdt, "w_sb")
        cc = [sb([128, rpp[i], D], cdt, f"cc{i}") for i in range(ncc)]
        xt_sb = sb([D, SQ_SHARD], cdt, "xt_sb")
        ut_sb = sb([D, D], cdt, "ut_sb")
        gs_m = sb([D, D], cdt, "gs_m")
        gs_l = sb([D, D], cdt, "gs_l")
        ps_sb = sb([D, D], cdt, "ps_sb")
        a_sb = sb([D, D], cdt, "a_sb")
        o_sb = sb([128, SQ_SHARD], cdt, "o_sb")

        g_ps = pst([128, 512], F32)  # b0 (use [:, :128])
        ut_ps = pst([128, 512], F32)  # b1: UT [:, :128], G_last [:, 128:256]
        p_ps = pst([128, 512], F32)  # b2
        a_ps = pst([128, 512], F32)  # b3
        o_ps = [pst([128, 512], F32) for _ in range(4)]  # b4..b7

        s_wa = sem("s_wa")  # wq|wk (sync)
        s_wb = sem("s_wb")  # wvt (scalar)
        s_x = sem("s_x")
        s_c = [sem(f"s_c{i}") for i in range(ncc)]
        s_pe = sem("s_pe")
        s_dve = sem("s_dve")
        s_o = [sem(f"s_o{k}") for k in range(4)]
        s_st = sem("s_st")

        pe = [0]

        def inc():
            pe[0] += 1
            return pe[0]

        with nc.Block() as block:

            @block.sync
            def _(sync):
                nc.sync.dma_start(w_sb[:, 0:256], w_ext[:, 0:256]).then_inc(
                    s_wa, 16
                )
                nc.sync.dma_start(cc[0][:], ctx_view[0]).then_inc(s_c[0], 16)
                nc.sync.dma_start(cc[2][:], ctx_view[2]).then_inc(s_c[2], 16)
                nc.sync.dma_start(cc[4][:], ctx_view[4]).then_inc(s_c[4], 16)
                nc.sync.wait_ge(s_o[0], 1)
                nc.sync.wait_ge(s_o[1], 1)
                nc.sync.dma_start(
                    outt_ext[:, 0:1024], o_sb[:, 0:1024]
                ).then_inc(s_st, 16)
                nc.sync.wait_ge(s_o[2], 1)
                nc.sync.wait_ge(s_o[3], 1)
                nc.sync.dma_start(
                    outt_ext[:, 1024:2048], o_sb[:, 1024:2048]
                ).then_inc(s_st, 16)
                # no s_st wait: the NEFF epilogue drains the DMA queues,
                # so the exit barrier need not stall on store-completion
                # semaphores (~2.4us latency after the data moves)

            @block.scalar
            def _(sc):
                nc.scalar.dma_start(
                    w_sb[:, 256:384], w_ext[:, 256:384]
                ).then_inc(s_wb, 16)
                nc.scalar.dma_start(cc[1][:], ctx_view[1]).then_inc(s_c[1], 16)
                nc.scalar.dma_start(cc[3][:], ctx_view[3]).then_inc(s_c[3], 16)
                nc.scalar.dma_start(cc[5][:], ctx_view[5]).then_inc(s_c[5], 16)
                nc.scalar.dma_start(xt_sb[:], xt_ext[:]).then_inc(s_x, 16)
                nc.scalar.wait_ge(s_pe, MARKS["o1"])
                nc.scalar.copy(
                    o_sb[:, 512:1024], o_ps[1][:]
                ).then_inc(s_o[1], 1)
                nc.scalar.wait_ge(s_pe, MARKS["o3"])
                nc.scalar.copy(
                    o_sb[:, 1536:2048], o_ps[3][:]
                ).then_inc(s_o[3], 1)

            @block.gpsimd
            def _(gp):
                pass

            @block.tensor
            def _(te):
                def filler(n, src):
                    # keep the PE busy across DMA gaps; results discarded
                    for i in range(n):
                        nc.tensor.matmul(
                            o_ps[i % 4][:, :128],
                            src,
                            src,
                            start=True,
                            stop=True,
                        ).then_inc(s_pe, 1)
                        inc()

                marks = {}
                # G over chunks 0..4 -> g_ps (G_main); chunk 5 -> ut_ps
                for c in range(ncc):
                    last_main = c == ncc - 2
                    is_last = c == ncc - 1
                    if is_last:
                        # b1 holds UT until the ut copy is done
                        nc.tensor.wait_ge(s_dve, 1)
                    nc.tensor.wait_ge(s_c[c], 16)
                    for j in range(rpp[c]):
                        dst = (
                            ut_ps[:, 128:256] if is_last else g_ps[:, :128]
                        )
                        nc.tensor.matmul(
                            dst,
                            cc[c][:, j, :],
                            cc[c][:, j, :],
                            start=(c == 0 and j == 0)
                            or (is_last and j == 0),
                            stop=(last_main and j == rpp[c] - 1)
                            or (is_last and j == rpp[c] - 1),
                        ).then_inc(s_pe, 1)
                        inc()
                    if c == 0:
                        # UT = Wk^T (scale*Wq), early (b1)
                        nc.tensor.wait_ge(s_wa, 16)
                        nc.tensor.matmul(
                            ut_ps[:, :128],
                            w_sb[:, 128:256],
                            w_sb[:, 0:128],
                            start=True,
                            stop=True,
                        ).then_inc(s_pe, 1)
                        marks["ut"] = inc()
                    if last_main:
                        marks["gmain"] = pe[0]
                    if is_last:
                        marks["glast"] = pe[0]
                    filler(FILLERS[c], cc[c][:, 0, :])

                # P = G Wv^T split into main+last accumulation (b2)
                nc.tensor.wait_ge(s_x, 16)
                nc.tensor.wait_ge(s_wb, 16)
                nc.tensor.wait_ge(s_dve, 2)
                nc.tensor.matmul(
                    p_ps[:, :128],
                    gs_m[:],
                    w_sb[:, 256:384],
                    start=True,
                    stop=False,
                ).then_inc(s_pe, 1)
                inc()
                nc.tensor.wait_ge(s_dve, 3)
                nc.tensor.matmul(
                    p_ps[:, :128],
                    gs_l[:],
                    w_sb[:, 256:384],
                    start=False,
                    stop=True,
                ).then_inc(s_pe, 1)
                marks["p"] = inc()
                # A = U P  (lhsT = U^T) (b3)
                nc.tensor.wait_ge(s_dve, 4)
                nc.tensor.matmul(
                    a_ps[:, :128], ut_sb[:], ps_sb[:], start=True, stop=True
                ).then_inc(s_pe, 1)
                marks["a"] = inc()
                # out^T = A^T X^T in 4 chunks of 512 query columns
                nc.tensor.wait_ge(s_dve, 5)
                for k in range(4):
                    nc.tensor.matmul(
                        o_ps[k][:],
                        a_sb[:],
                        xt_sb[:, 512 * k : 512 * (k + 1)],
                        start=True,
                        stop=True,
                    ).then_inc(s_pe, 1)
                    marks[f"o{k}"] = inc()
                assert marks == MARKS, (marks, MARKS)

            @block.vector
            def _(ve):
                nc.vector.wait_ge(s_pe, MARKS["ut"])
                nc.vector.tensor_copy(ut_sb[:], ut_ps[:, :128]).then_inc(
                    s_dve, 1
                )
                nc.vector.wait_ge(s_pe, MARKS["gmain"])
                nc.vector.tensor_copy(gs_m[:], g_ps[:, :128]).then_inc(
                    s_dve, 1
                )
                nc.vector.wait_ge(s_pe, MARKS["glast"])
                nc.vector.tensor_copy(gs_l[:], ut_ps[:, 128:256]).then_inc(
                    s_dve, 1
                )
                nc.vector.wait_ge(s_pe, MARKS["p"])
                nc.vector.tensor_copy(ps_sb[:], p_ps[:, :128]).then_inc(
                    s_dve, 1
                )
                nc.vector.wait_ge(s_pe, MARKS["a"])
                nc.vector.tensor_copy(a_sb[:], a_ps[:, :128]).then_inc(
                    s_dve, 1
                )
                nc.vector.wait_ge(s_pe, MARKS["o0"])
                nc.vector.tensor_copy(o_sb[:, 0:512], o_ps[0][:]).then_inc(
                    s_o[0], 1
                )
                nc.vector.wait_ge(s_pe, MARKS["o2"])
                nc.vector.tensor_copy(
                    o_sb[:, 1024:1536], o_ps[2][:]
                ).then_inc(s_o[2], 1)

    nc.compile()
    return nc


def build():
    return build_v4()


def _get_nc():
    if "nc" not in _CACHE:
        _CACHE["nc"] = build()
    return _CACHE["nc"]


def _run(inputs: dict, trace: bool = False, **kw):
    np_dt = ml_dtypes.bfloat16
    context = np.ascontiguousarray(inputs["context"]).astype(np_dt)
    Wq = np.asarray(inputs["Wq"], dtype=np.float32) * SCALE
    Wk = np.asarray(inputs["Wk"], dtype=np.float32)
    Wvt = np.asarray(inputs["Wv"], dtype=np.float32).T
    w_pack = np.ascontiguousarray(
        np.concatenate([Wq, Wk, Wvt], axis=1)
    ).astype(np_dt)
    X = np.asarray(inputs["X"], dtype=np.float32)

    in_maps = []
    for c in range(N_CORES):
        b, h = divmod(c, 2)
        xt = np.ascontiguousarray(
            X[b, h * SQ_SHARD : (h + 1) * SQ_SHARD, :].T
        ).astype(np_dt)
        in_maps.append({"xt": xt, "ctx": context[b], "w": w_pack})

    nc = _get_nc()
    res = run_bass_kernel_spmd(
        nc, in_maps, core_ids=list(range(N_CORES)), trace=trace, **kw
    )
    out = np.empty((B, SQ, D), dtype=np.float32)
    for c in range(N_CORES):
        b, h = divmod(c, 2)
        out[b, h * SQ_SHARD : (h + 1) * SQ_SHARD, :] = (
            res.results[c]["outt"].astype(np.float32).T
        )
    return out, res


def kernel(**inputs: np.ndarray) -> np.ndarray:
    if os.environ.get("BASS_TRACE"):
        _install_axon_ntff_shim()
    try:
        out, _ = _run(inputs, trace=False)
    except Exception:
        # transient NRT device errors have been observed once across many
        # runs; one retry on a fresh execution
        out, _ = _run(inputs, trace=False)
    return out


if __name__ == "__main__":
    rng = np.random.default_rng(0)
    ins = {
        "context": rng.standard_normal((B, SKV, D)).astype(np.float32),
        "X": rng.standard_normal((B, SQ, D)).astype(np.float32),
        "Wq": (rng.standard_normal((D, D)) / np.sqrt(D)).astype(np.float32),
        "Wk": (rng.standard_normal((D, D)) / np.sqrt(D)).astype(np.float32),
        "Wv": (rng.standard_normal((D, D)) / np.sqrt(D)).astype(np.float32),
    }
    got = kernel(**ins)
    q = ins["X"] @ ins["Wq"].T
    k = ins["context"] @ ins["Wk"].T
    v = ins["context"] @ ins["Wv"].T
    w = np.einsum("bse,bte->bst", q, k) * SCALE
    want = np.einsum("bst,bte->bse", w, v)
    rel = np.linalg.norm(got - want) / np.linalg.norm(want)
    print("rel err vs numpy:", rel)
